# revision 35
# baseline (speedup 1.0000x reference)
"""Trainium2 Bass kernel for GQA attention with RoPE, causal mask, and
attention sinks (nn_Attention_65094524338392).

Sharding: tensor-parallel by heads across 8 NeuronCores. Core c owns query
heads 4c..4c+3 and kv-head c (NREP=4). Each core computes QKV projections
over the full sequence for its heads, flash-style causal attention, then a
head-pair-chunked AllToAll redistributes attention outputs from
head-sharding to sequence-sharding so each core computes the output
projection for its 256-row sequence slice.

Key optimizations over the v1 baseline:
 - x is transposed on the host; the kernel loads x^T with plain DMAs
   (the v1 DMA-transposes serialized ~81us of descriptor-gen on SyncE).
 - exp() is split across the Scalar engine (table exp) and the
   Vector+GpSimd engines (Schraudolph bit-trick exp: int32 affine with
   output conversion, then a bitcast copy to bf16), so softmax is no
   longer serialized on one engine.
 - sink/softmax normalization is batched per head-pair ([128,2048] ops)
   instead of per (block, head).
 - the AllToAll is split into two per-head-pair chunks; chunk 0 overlaps
   the second half of attention, and the output projection is split into
   even/odd contraction halves so it starts during chunk 1.
 - woT streams in during attention (gpsimd queue program order) instead
   of competing with x/wqkv for HBM at startup.

Math note: the sink scaling folds into the softmax normalizer:
    out = (sum_k exp(s_k) v_k) / (sum_k exp(s_k) + exp(sink))
so no logs/sigmoids are needed on device; exp(sink) is precomputed on the
host. Because |s| <= ~40, no max-subtraction is needed for fp32 exp.
"""

import os
import sys

sys.path.insert(0, "/opt/trn_rl_repo")

import ml_dtypes
import numpy as np

import concourse.bass as bass
import concourse.mybir as mybir
import concourse.tile as tile
from concourse import bacc
from concourse.bass_utils import run_bass_kernel_spmd

# Problem shapes
B, S, DIM = 1, 2048, 2048
NH, NKV, HD = 32, 8, 64
NREP = NH // NKV
SCALE = 1.0 / float(np.sqrt(HD))
NCORES = 8
HPC = NH // NCORES            # query heads per core (4)
QKV = HPC * HD + 2 * HD       # fused qkv output dim per core (384)
QW = HPC * HD                 # query width per core (256)
SB = 512                      # seq block (attention q-block)
NSB = S // SB                 # 4
NT = S // 128                 # 16 seq tiles
ND = DIM // 128               # 16 contraction tiles
MYS = S // NCORES             # output rows per core (256)

F32 = mybir.dt.float32
BF16 = mybir.dt.bfloat16
I16 = mybir.dt.int16
F8 = mybir.dt.float8e4

# bf16 Schraudolph exp constants: exp(x) ~= bitcast_bf16(int16(A*x + B)).
# One DVE pass: the fp32->int16 output conversion rounds the affine result
# directly into bf16's bit layout (max rel err ~3.3%).
S16_A = float(128.0 / np.log(2.0))
S16_B = float(127 * 128 - 5.5)

_cache = {}

last_exec_time_ns = None


def _install_ntff_shim():
    """Register the NTFF profile hook so trace=True yields exec_time_ns."""
    import types
    if "antenv.axon_hooks" in sys.modules:
        return
    import antenv
    mod = types.ModuleType("antenv.axon_hooks")
    mod._hook = None
    mod.set_axon_ntff_profile_hook = lambda h: setattr(mod, "_hook", h)
    mod.get_axon_ntff_profile_hook = lambda: mod._hook
    sys.modules["antenv.axon_hooks"] = mod
    antenv.axon_hooks = mod
    from trn_agent_boot.trn_boot import _ntff_profile_via_ctypes
    hook = _ntff_profile_via_ctypes("/opt/axon/libaxon_pjrt.so")
    if hook is not None:
        mod._hook = hook


def _build():
    nc = bacc.Bacc("TRN2", target_bir_lowering=False, debug=False,
                   num_devices=NCORES)

    # Input staging happens in declaration order; order by first use:
    # qkv weights + x^T feed phase B immediately, rope consts next,
    # attention consts after, woT (phase D) last.
    qkvb_e = nc.declare_dram_parameter("qkvb", [1, QKV], BF16, isOutput=False)
    wqkvT_e = nc.declare_dram_parameter("wqkvT", [128, ND * QKV], BF16, isOutput=False)
    xT_e = nc.declare_dram_parameter("xT", [128, NSB * ND * SB], BF16, isOutput=False)
    cosd_e = nc.declare_dram_parameter("cosd", [128, NT * HD], F32, isOutput=False)
    nsin_e = nc.declare_dram_parameter("nsin", [128, NT * HD // 2], F32, isOutput=False)
    psin_e = nc.declare_dram_parameter("psin", [128, NT * HD // 2], F32, isOutput=False)
    ident_e = nc.declare_dram_parameter("ident", [128, 128], BF16, isOutput=False)
    mask_e = nc.declare_dram_parameter("masks", [128, 4 * SB], BF16, isOutput=False)
    esg_e = nc.declare_dram_parameter("esg", [8, 2], F32, isOutput=False)
    wob_e = nc.declare_dram_parameter("wob", [1, DIM], BF16, isOutput=False)
    woT_e = nc.declare_dram_parameter("woT", [128, ND * DIM], BF16, isOutput=False)
    out_e = nc.declare_dram_parameter("out", [MYS, DIM], F32, isOutput=True)

    with tile.TileContext(nc) as tc:
        with tc.tile_pool(name="const", bufs=1) as cp, \
             tc.tile_pool(name="rope", bufs=2) as rp, \
             tc.tile_pool(name="qkr", bufs=4) as qkrp, \
             tc.tile_pool(name="pt", bufs=5) as ptp, \
             tc.tile_pool(name="it", bufs=3) as itp, \
             tc.tile_pool(name="ep", bufs=3) as epp, \
             tc.tile_pool(name="fin", bufs=2) as fnp, \
             tc.tile_pool(name="dram", bufs=1, space="DRAM") as dp:

            # ---- staging (sync queue is FIFO: order = priority) ----
            qkvb_sb = cp.tile([1, QKV], BF16)
            nc.sync.dma_start(qkvb_sb[:], qkvb_e[:])
            wqkvT_sb = cp.tile([128, ND, QKV], BF16)
            wqv = wqkvT_e[:].rearrange("p (o f) -> p o f", o=ND)
            nc.sync.dma_start(wqkvT_sb[:, 0:ND // 2, :], wqv[:, 0:ND // 2, :])
            nc.sync.dma_start(wqkvT_sb[:, ND // 2:, :], wqv[:, ND // 2:, :])
            # x^T view, half-block-major so each DMA is one contiguous
            # 8KB-per-partition descriptor: [128, s, half, d, c]
            xTv = xT_e[:].rearrange("p (s h d c) -> p s h d c",
                                    s=NSB, h=2, d=ND)
            # two fixed x^T half-block buffers, alternated: reusing the SAME
            # tiles forces WAR pacing so later half-block DMAs cannot race
            # ahead and steal startup DMA bandwidth from the critical loads
            xt2 = [cp.tile([128, ND, SB // 2], BF16, name=f"xt2_{i}")
                   for i in range(2)]
            nc.sync.dma_start(xt2[0][:, 0:ND // 2, :],
                              xTv[:, 0, 0, 0:ND // 2, :])
            nc.sync.dma_start(xt2[0][:, ND // 2:, :],
                              xTv[:, 0, 0, ND // 2:, :])
            # DMA engines round-robin every in-flight transfer, so gate the
            # non-critical loads on xt2[0]'s data: the first QKV consumer
            # then gets the full DMA bandwidth
            cos_sb = cp.tile([128, NT, HD], F32)
            nsin_sb = cp.tile([128, NT, HD // 2], F32)
            psin_sb = cp.tile([128, NT, HD // 2], F32)
            ident_sb = cp.tile([128, 128], BF16)
            gate_src = xt2[0][0:1, 0, 0:1]
            nc.vector.tensor_copy(xt2[1][0:1, 0, 0:1], gate_src)
            nc.vector.tensor_copy(cos_sb[0:1, 0, 0:1], gate_src)
            nc.vector.tensor_copy(nsin_sb[0:1, 0, 0:1], gate_src)
            nc.vector.tensor_copy(psin_sb[0:1, 0, 0:1], gate_src)
            nc.vector.tensor_copy(ident_sb[0:1, 0:1], gate_src)
            nc.sync.dma_start(xt2[1][:], xTv[:, 0, 1])
            nc.sync.dma_start(cos_sb[:], cosd_e[:].rearrange(
                "p (o f) -> p o f", o=NT))
            nc.sync.dma_start(nsin_sb[:], nsin_e[:].rearrange(
                "p (o f) -> p o f", o=NT))
            nc.sync.dma_start(psin_sb[:], psin_e[:].rearrange(
                "p (o f) -> p o f", o=NT))
            nc.sync.dma_start(ident_sb[:], ident_e[:])

            woT_sb = cp.tile([128, ND, DIM], BF16)
            ones_sb = cp.tile([1, 128], BF16)
            nc.gpsimd.memset(ones_sb[:], 1.0)

            # PE warm-up: a gapless burst of dummy matmuls during the
            # initial input DMAs releases the HAM clock throttle before
            # real work begins.
            warm_sb = cp.tile([128, 512], BF16)
            nc.gpsimd.memset(warm_sb[:], 0.0)
            with tc.tile_pool(name="warm", bufs=1, space="PSUM") as wpp:
                warm_ps = wpp.tile([128, 512], F32, tag="warm")
                for _ in range(10):
                    nc.tensor.matmul(warm_ps[:], warm_sb[:, 0:128],
                                     warm_sb[:], start=True, stop=True)

            # persistent activations: q head-pairs packed in partition
            # halves, k duplicated into both halves (tile_position packing)
            qP = [cp.tile([128, S], BF16, name=f"qP{g}") for g in range(HPC // 2)]
            kTd = cp.tile([128, S], BF16)
            v_sb = cp.tile([128, NT, HD + 1], BF16)
            nc.gpsimd.memset(v_sb[:, :, HD:HD + 1], 1.0)
            # attention outputs per head: rows 0..HD-1 hold the (unnormalized
            # until the epilogue) PV output, row HD holds the softmax row sum
            oT5 = [cp.tile([HD + 1, S], BF16, name=f"oT5{h}")
                   for h in range(HPC)]
            # row-sum collectors (one per head pair): row r = 2*s + z
            srg8 = [cp.tile([8, SB], BF16, name=f"srg8_{g}")
                    for g in range(2)]
            rgb1 = cp.tile([1, 8 * SB], BF16)
            rbcz = [cp.tile([HD, S], BF16, name=f"rbcz{z}") for z in range(2)]
            # A2A gather tiles, 4 contraction strips each (phase D weights)
            agh = [[cp.tile([128, 4, MYS], BF16, name=f"ag{g}_{h}")
                    for h in range(2)] for g in range(2)]

            a2a_in = [dp.tile([S // 2, MYS], BF16, name=f"a2ai{g}")
                      for g in range(2)]
            a2a_out = [dp.tile([S // 2, MYS], BF16, name=f"a2ao{g}")
                       for g in range(2)]

            # ---- phase B: QKV projections + rope + transposes ----
            with tc.tile_pool(name="ppB", bufs=2, space="PSUM") as ppB:
                first_warm = [True]
                for s in range(NSB):
                    qkr_tiles = []
                    for tt in range(4):
                        t = 4 * s + tt
                        hb = 2 * s + tt // 2
                        if tt % 2 == 0 and hb >= 2:
                            # refill the alternating half-block buffer (the
                            # first two DMAs were issued at staging time)
                            nc.sync.dma_start(xt2[hb % 2][:],
                                              xTv[:, hb // 2, hb % 2])
                        xt = xt2[hb % 2]
                        xsl = (tt % 2) * 128

                        if first_warm[0]:
                            # dependency-gated warm-up: fires as soon as the
                            # first x^T tile lands, so QKV starts at the warm
                            # PE clock
                            first_warm[0] = False
                            warm2_ps = ppB.tile([128, 512], F32, tag="warmB",
                                                bufs=1)
                            for _ in range(12):
                                nc.tensor.matmul(warm2_ps[:],
                                                 xt[:, 0, 0:128],
                                                 warm_sb[:], start=True,
                                                 stop=True)

                        # fused qkv projection for this seq tile
                        acc = ppB.tile([128, QKV], F32, tag="acc", bufs=3)
                        for d in range(ND):
                            nc.tensor.matmul(acc[:],
                                             xt[:, d, xsl:xsl + 128],
                                             wqkvT_sb[:, d, :],
                                             start=(d == 0), stop=False)
                        nc.tensor.matmul(acc[:], ones_sb[0:1, :], qkvb_sb[:],
                                         start=False, stop=True)

                        # rope on q and k halves (free-dim ops, 5 = 4q+1k)
                        W = QW + HD  # 320
                        tmp = rp.tile([128, W], F32, tag="tmp")
                        qkr = qkrp.tile([128, W], BF16, tag="qkr")
                        acc5 = acc[:, 0:W].rearrange("p (h x) -> p h x", x=HD)
                        tmp5 = tmp[:].rearrange("p (h x) -> p h x", x=HD)
                        nc.vector.tensor_tensor(
                            tmp5[:, :, 0:HD // 2], acc5[:, :, HD // 2:HD],
                            nsin_sb[:, t:t + 1, :].to_broadcast(
                                [128, 5, HD // 2]),
                            mybir.AluOpType.mult)
                        nc.vector.tensor_tensor(
                            tmp5[:, :, HD // 2:HD], acc5[:, :, 0:HD // 2],
                            psin_sb[:, t:t + 1, :].to_broadcast(
                                [128, 5, HD // 2]),
                            mybir.AluOpType.mult)
                        nc.vector.tensor_tensor(
                            qkr[:], acc[:, 0:W],
                            cos_sb[:, t:t + 1, :].to_broadcast([128, 5, HD]),
                            mybir.AluOpType.mult)
                        nc.vector.tensor_tensor(qkr[:], qkr[:], tmp[:],
                                                mybir.AluOpType.add)
                        qkr_tiles.append(qkr)
                        # v: plain copy (cast to bf16)
                        nc.scalar.copy(v_sb[:, t, 0:HD], acc[:, QW + HD:QKV])

                    # transpose rope'd q/k for the block into [hd, seq]
                    for h in range(HPC + 1):
                        tq_ps = ppB.tile([HD, 512], BF16, tag="tq", bufs=2)
                        for tt in range(4):
                            nc.tensor.transpose(
                                tq_ps[:, tt * 128:(tt + 1) * 128],
                                qkr_tiles[tt][:, h * HD:(h + 1) * HD],
                                ident_sb[:])
                        sl = slice(s * SB, (s + 1) * SB)
                        if h < HPC:
                            dst = qP[h // 2][(h % 2) * HD:(h % 2) * HD + HD, sl]
                            if h % 2 == 0:
                                nc.scalar.copy(dst, tq_ps[:])
                            else:
                                nc.vector.tensor_copy(dst, tq_ps[:])
                        else:
                            nc.scalar.copy(kTd[0:HD, sl], tq_ps[:])
                            nc.vector.tensor_copy(kTd[HD:2 * HD, sl], tq_ps[:])

            # attention/output consts: needed only after phase B
            mask_sb = cp.tile([128, 4, SB], BF16)
            nc.sync.dma_start(mask_sb[:], mask_e[:].rearrange(
                "p (d f) -> p d f", d=4))
            esg_sb = cp.tile([8, 2], F32)
            nc.sync.dma_start(esg_sb[:], esg_e[:])
            wob_sb = cp.tile([1, DIM], BF16)
            nc.sync.dma_start(wob_sb[:], wob_e[:])

            # ---- phase C: attention (flash-style over causal k-tiles) ----
            # Scores for a head PAIR run concurrently on the two 64-row
            # halves of the PE array (tile_position row packing). Each k-tile
            # produces one [128, (z, 512)] fp32 score tile; exp/mask run as
            # single wide ops covering both heads; PV matmuls trail the
            # score stream by two k-tiles so the exp latency stays hidden.
            exp_counter = [0]
            mask_counter = [0]

            def emit_sums(g, s_lo, s_hi):
                # reciprocal softmax normalizers for seq blocks s_lo..s_hi-1
                # of head pair g, broadcast into rbcz via DRAM
                r0, r1 = 2 * s_lo, 2 * s_hi
                c0, c1 = s_lo * SB, s_hi * SB
                ns = s_hi - s_lo
                # engine APs must start 32-aligned, so compute on all 8 rows
                # (DVE cost depends on free size only) and slice the DMA
                srgf = epp.tile([8, SB], F32, tag="srgf", bufs=2)
                nc.vector.tensor_scalar(
                    srgf[:], srg8[g][:], esg_sb[:, g:g + 1],
                    None, mybir.AluOpType.add)
                rr = epp.tile([8, SB], F32, tag="rr", bufs=2)
                nc.vector.reciprocal_approx_fast(rr[:], srgf[:])
                rgb = epp.tile([8, SB], BF16, tag="rgb", bufs=2)
                nc.vector.tensor_copy(rgb[:], rr[:])
                # flatten rows r0:r1 into rgb1 cols (row-major: (s, z, c))
                nc.sync.dma_start(rgb1[0:1, r0 * SB:r1 * SB], rgb[r0:r1, :])
                rg1v = rgb1[0:1, r0 * SB:r1 * SB].rearrange(
                    "a (s z c) -> a z s c", z=2, c=SB)
                for z in range(2):
                    # GPS queue is empty at epilogue time, so the one-hop
                    # partition_broadcast beats a DRAM round trip
                    nc.gpsimd.partition_broadcast(
                        rbcz[z][:, c0:c1].rearrange("p (s c) -> p s c", c=SB),
                        rg1v[:, z])

            def emit_norm(g, s_lo, s_hi):
                # normalize + scatter seq blocks s_lo..s_hi-1 of head pair g
                c0, c1 = s_lo * SB, s_hi * SB
                ns = s_hi - s_lo
                for z in range(2):
                    h = 2 * g + z
                    nc.vector.tensor_tensor(oT5[h][0:HD, c0:c1],
                                            oT5[h][0:HD, c0:c1],
                                            rbcz[z][:, c0:c1],
                                            mybir.AluOpType.mult)
                    nc.sync.dma_start(
                        a2a_in[g][:].rearrange(
                            "(j two p) n -> two p j n",
                            j=NCORES, two=2)[z][:, 2 * s_lo:2 * s_hi, :],
                        oT5[h][0:HD, c0:c1].rearrange(
                            "p (j n) -> p j n", j=2 * ns))

            def emit_chunk(g):
                nc.gpsimd.collective_compute(
                    "AllToAll", mybir.AluOpType.bypass,
                    replica_groups=[list(range(NCORES))],
                    ins=[a2a_in[g].opt()], outs=[a2a_out[g].opt()])

            def emit_gathers(g):
                # phase D gather tiles ride the (otherwise idle during g1)
                # GpSimd queue: parking there blocks nothing
                a2o = a2a_out[g][:].rearrange("(c p) n -> c p n", p=128)
                for h in range(2):
                    nc.gpsimd.dma_start(
                        agh[g][h][:],
                        a2o[4 * h:4 * h + 4].rearrange("c p n -> p c n"))

            with tc.tile_pool(name="ppC", bufs=1, space="PSUM") as ppC:
                for g in range(HPC // 2):
                    # s descending: the tail block (last before the A2A
                    # chunk fires) is the smallest one
                    for s in reversed(range(NSB)):
                        n_kt = 4 * (s + 1)
                        sl = slice(s * SB, (s + 1) * SB)
                        pv = [ppC.tile([HD + 1, 512], F32, tag=f"pv{z}",
                                       bufs=1, name=f"pv{g}_{s}_{z}")
                              for z in range(2)]
                        pts = []

                        def emit_pv(i0, i1):
                            # PV for k-tiles i0..i1 (inclusive), grouped by
                            # PSUM bank (z)
                            for z in range(2):
                                for ii in range(i0, i1 + 1):
                                    nc.tensor.matmul(
                                        pv[z][:], v_sb[:, ii, :],
                                        pts[ii][:, z, :],
                                        start=(ii == 0),
                                        stop=(ii == n_kt - 1))

                        for i in range(n_kt):
                            sc = ppC.tile([128, 2, 512], F32, tag="sc",
                                          bufs=3)
                            for z in range(2):
                                nc.tensor.matmul(
                                    sc[:, z, :],
                                    kTd[z * HD:(z + 1) * HD,
                                        i * 128:(i + 1) * 128],
                                    qP[g][z * HD:(z + 1) * HD, sl],
                                    start=True, stop=True,
                                    tile_position=(z * HD, 0))
                            cnt = exp_counter[0]
                            exp_counter[0] += 1
                            if (cnt % 8) in (0, 1, 3, 4, 6):
                                pt = ptp.tile([128, 2, 512], BF16, tag="pt")
                                nc.scalar.activation(
                                    pt[:], sc[:],
                                    mybir.ActivationFunctionType.Exp,
                                    scale=SCALE)
                                ptv = pt[:]
                            else:
                                # single-pass bf16 Schraudolph exp: the
                                # int16 affine result IS the bf16 bits
                                it_ = itp.tile([128, 2, 512], I16, tag="it")
                                nc.vector.tensor_scalar(
                                    it_[:], sc[:], S16_A * SCALE, S16_B,
                                    mybir.AluOpType.mult,
                                    mybir.AluOpType.add)
                                ptv = it_[:].bitcast(BF16)
                            if i >= 4 * s:
                                # causal mask for a diagonal k-tile, one op
                                # covering both heads; mostly DVE, some GPS
                                d = i - 4 * s
                                mc = mask_counter[0]
                                mask_counter[0] += 1
                                # GPS masks only during g0: in g1 the GPS
                                # FIFO must stay clear for the A2A triggers
                                eng = (nc.gpsimd
                                       if (g == 0 and s != 3
                                           and (mc % 4) == 0)
                                       else nc.vector)
                                eng.tensor_tensor(
                                    ptv, ptv,
                                    mask_sb[:, d:d + 1, :].to_broadcast(
                                        [128, 2, 512]),
                                    mybir.AluOpType.mult)
                            pts.append(ptv)
                            # PV trails by 2 k-tiles so exp latency is
                            # covered by the next scores
                            if i >= 3 and i % 2 == 1:
                                emit_pv(i - 3, i - 2)
                        emit_pv(n_kt - 2, n_kt - 1)

                        for z in range(2):
                            # pv -> persistent bf16 output rows (incl. the
                            # sum row at partition HD); sum row then hops to
                            # the srg8 collector via DMA
                            h = 2 * g + z
                            if z == 0:
                                nc.scalar.copy(oT5[h][:, sl], pv[z][:])
                            else:
                                nc.vector.tensor_copy(oT5[h][:, sl], pv[z][:])
                            nc.sync.dma_start(
                                srg8[g][2 * s + z:2 * s + z + 1, :],
                                oT5[h][HD:HD + 1, sl])

                        if g == 0 and s == 3:
                            # stream woT in now: HBM is otherwise idle during
                            # attention and phase D needs it much later. The
                            # gate copy adds a dependency on attention output
                            # so the scheduler cannot hoist the load into the
                            # startup HBM rush.
                            nc.gpsimd.tensor_copy(
                                woT_sb[0:1, 0, 0:1],
                                oT5[2 * g][0:1, 3 * SB:3 * SB + 1])
                            nc.gpsimd.dma_start(
                                woT_sb[:, 0:ND // 2, :],
                                woT_e[:, 0:ND // 2 * DIM].rearrange(
                                    "p (o f) -> p o f", o=ND // 2))
                        if g == 0 and s == 2:
                            nc.gpsimd.tensor_copy(
                                woT_sb[0:1, ND // 2, 0:1],
                                oT5[2 * g][0:1, 2 * SB:2 * SB + 1])
                            nc.gpsimd.dma_start(
                                woT_sb[:, ND // 2:, :],
                                woT_e[:, ND // 2 * DIM:].rearrange(
                                    "p (o f) -> p o f", o=ND // 2))
                        # pair-0 epilogues are deferred into g1's blocks so
                        # their semaphore waits never park an engine FIFO
                        # ahead of available attention work; pair-1's batchA
                        # chain starts one block early for the same reason
                        if g == 1 and s == 3:
                            emit_sums(0, 1, NSB)
                            emit_norm(0, 1, NSB)
                        if g == 1 and s == 2:
                            emit_sums(0, 0, 1)
                            emit_norm(0, 0, 1)
                            emit_chunk(0)
                        if g == 1 and s == 1:
                            emit_sums(1, 1, NSB)
                            emit_gathers(0)
                # pair-1 epilogue tail: the last chain gates the final A2A
                emit_norm(1, 1, NSB)
                emit_sums(1, 0, 1)
                emit_norm(1, 0, 1)
                emit_chunk(1)
                emit_gathers(1)

            # ---- output projection for my sequence slice ----
            # even contraction tiles (chunk 0) first so they can run while
            # A2A chunk 1 is still in flight
            with tc.tile_pool(name="ppD", bufs=1, space="PSUM") as ppD:
                fps = [ppD.tile([128, 512], F32, name=f"fp{m}_{n}", bufs=1)
                       for m in range(MYS // 128) for n in range(DIM // 512)]
                for parity in range(2):
                    for m in range(MYS // 128):
                        for n in range(DIM // 512):
                            fp = fps[m * (DIM // 512) + n]
                            for c in range(ND // 2):
                                kt = 2 * c + parity
                                nc.tensor.matmul(
                                    fp[:],
                                    agh[parity][c // 4][:, c % 4,
                                                        m * 128:(m + 1) * 128],
                                    woT_sb[:, kt, n * 512:(n + 1) * 512],
                                    start=(c == 0 and parity == 0),
                                    stop=(parity == 1 and c == ND // 2 - 1))
                            if parity == 0:
                                # bias needs no A2A data: accumulate it in
                                # the first wave, off the post-collective
                                # critical path
                                nc.tensor.matmul(
                                    fp[:], ones_sb[0:1, :],
                                    wob_sb[0:1, n * 512:(n + 1) * 512],
                                    start=False, stop=False)
                            if parity == 1:
                                fo = fnp.tile([128, 512], F32, tag="fo")
                                if (m * 4 + n) % 2 == 0:
                                    nc.scalar.copy(fo[:], fp[:])
                                else:
                                    nc.vector.tensor_copy(fo[:], fp[:])
                                nc.sync.dma_start(
                                    out_e[m * 128:(m + 1) * 128,
                                          n * 512:(n + 1) * 512], fo[:])

    nc.compile()
    return nc


def _host_prep(x, rope_cache, wq_w, wq_b, wk_w, wk_b, wv_w, wv_b,
               wo_w, wo_b, sinks):
    """Build the per-core input maps (sharding + layout prep)."""
    # x^T, half-block-major: [128, (s half d c)] with s=block, half=half-
    # block, d=dim-strip, c=col — each half-block contiguous per partition
    xT = np.asarray(x, np.float32).reshape(S, DIM).T.astype(
        ml_dtypes.bfloat16)
    xTb = np.ascontiguousarray(
        xT.reshape(ND, 128, NSB, 2, SB // 2).transpose(1, 2, 3, 0, 4).reshape(
            128, NSB * ND * SB))

    def _pm(a):
        # [S, F] -> [128, (S//128) * F] partition-major packing
        f = a.shape[1]
        return np.ascontiguousarray(
            a.reshape(S // 128, 128, f).transpose(1, 0, 2).reshape(
                128, (S // 128) * f))

    cos = np.asarray(rope_cache[:, :HD // 2], np.float32)
    sin = np.asarray(rope_cache[:, HD // 2:], np.float32)
    cosd = _pm(np.concatenate([cos, cos], axis=1))
    nsin = _pm(-sin)
    psin = _pm(sin)
    # causal masks for the 4 diagonal 128-row k-tiles of a 512-col q block
    masks = np.zeros((4, 128, SB), np.float32)
    for d in range(4):
        for p in range(128):
            masks[d, p, d * 128 + p:] = 1.0
    masks = np.ascontiguousarray(
        masks.transpose(1, 0, 2).reshape(128, 4 * SB)).astype(
            ml_dtypes.bfloat16)
    ident = np.eye(128, dtype=ml_dtypes.bfloat16)
    woT = np.asarray(wo_w, np.float32).T.astype(ml_dtypes.bfloat16)
    woT = np.ascontiguousarray(
        woT.reshape(ND, 128, DIM).transpose(1, 0, 2).reshape(128, ND * DIM))
    wob = np.asarray(wo_b, np.float32).astype(
        ml_dtypes.bfloat16).reshape(1, DIM)
    es_all = np.exp(np.asarray(sinks, np.float64)).astype(np.float32)

    in_maps = []
    for c in range(NCORES):
        qsl = slice(c * QW, (c + 1) * QW)
        ksl = slice(c * HD, (c + 1) * HD)
        wqkvT = np.concatenate([
            np.asarray(wq_w, np.float32)[qsl].T,
            np.asarray(wk_w, np.float32)[ksl].T,
            np.asarray(wv_w, np.float32)[ksl].T],
            axis=1).astype(ml_dtypes.bfloat16)
        wqkvT = np.ascontiguousarray(
            wqkvT.reshape(ND, 128, QKV).transpose(1, 0, 2).reshape(
                128, ND * QKV))
        qkvb = np.ascontiguousarray(np.concatenate([
            np.asarray(wq_b, np.float32)[qsl],
            np.asarray(wk_b, np.float32)[ksl],
            np.asarray(wv_b, np.float32)[ksl]]).astype(
                ml_dtypes.bfloat16)).reshape(1, QKV)
        # esg[2s+z, g] = exp(sink[4c + 2g + z]) (replicated over s)
        esg = np.zeros((8, 2), np.float32)
        for gg in range(2):
            for z in range(2):
                esg[z::2, gg] = es_all[4 * c + 2 * gg + z]
        in_maps.append({
            "xT": xTb, "wqkvT": wqkvT, "qkvb": qkvb, "cosd": cosd,
            "nsin": nsin, "psin": psin, "masks": masks, "ident": ident,
            "woT": woT, "wob": wob, "esg": esg,
        })
    return in_maps


def kernel(**inputs):
    global last_exec_time_ns
    if "nc" not in _cache:
        _cache["nc"] = _build()
    nc = _cache["nc"]
    in_maps = _host_prep(**inputs)
    trace = bool(int(os.environ.get("BASS_KERNEL_TRACE", "0")))
    if trace:
        try:
            _install_ntff_shim()
        except Exception:
            trace = False
    tc_env = os.environ.get("BASS_KERNEL_TRACE_CORES")
    kw = {}
    if trace and tc_env:
        kw["trace_cores"] = [int(c) for c in tc_env.split(",")]
    res = run_bass_kernel_spmd(nc, in_maps, core_ids=list(range(NCORES)),
                               trace=trace, **kw)
    last_exec_time_ns = res.exec_time_ns
    out = np.concatenate([res.results[c]["out"] for c in range(NCORES)],
                         axis=0)
    return out.reshape(B, S, NH * HD)



# revision 36
# speedup vs baseline: 1.1845x; 1.1845x over previous
"""Trainium2 Bass kernel for GQA attention with RoPE, causal mask, and
attention sinks (nn_Attention_65094524338392).

Sharding: tensor-parallel by heads across 8 NeuronCores. Core c owns query
heads 4c..4c+3 and kv-head c (NREP=4). Each core computes QKV projections
over the full sequence for its heads, flash-style causal attention, then a
head-pair-chunked AllToAll redistributes attention outputs from
head-sharding to sequence-sharding so each core computes the output
projection for its 256-row sequence slice.

Key optimizations over the v1 baseline:
 - x is transposed on the host; the kernel loads x^T with plain DMAs
   (the v1 DMA-transposes serialized ~81us of descriptor-gen on SyncE).
 - exp() is split 5:3 between the Scalar engine (table exp) and the
   Vector engine (Schraudolph int16 bit-trick exp); each op covers BOTH
   packed heads of a pair ([128, 2x512]), halving op count, and the
   causal mask is one broadcast tensor_tensor per diagonal k-tile.
 - phase C runs per k-tile: z-packed score matmuls (tile_position row
   packing, concurrent quadrants), exp, then PV matmuls trailing two
   k-tiles behind so the exp latency hides under later scores.
 - PV epilogue writes straight into persistent [65, S] output tiles
   (rows 0..63 = head dims, row 64 = softmax row sum); only the sum row
   hops partitions via DMA into a [8, 512] collector for batched
   normalizer math.
 - A2A chunk per head pair; pair-0's normalize/scatter/trigger work is
   deferred INTO pair-1's block stream so no engine FIFO ever parks on a
   not-yet-ready semaphore ahead of available attention work (engine
   queues are strict FIFO; a parked DMA/op stalls everything behind it).
 - phase D gathers ride the GpSimd queue (idle during pair 1) at
   half-chunk granularity; output projection split into even/odd
   contraction parities so parity 0 runs while chunk 1 is in flight.
 - woT streams in during attention (gpsimd queue program order) instead
   of competing with x/wqkv for HBM at startup.

Math note: the sink scaling folds into the softmax normalizer:
    out = (sum_k exp(s_k) v_k) / (sum_k exp(s_k) + exp(sink))
so no logs/sigmoids are needed on device; exp(sink) is precomputed on the
host. Because |s| <= ~40, no max-subtraction is needed for fp32 exp.
"""

import os
import sys

sys.path.insert(0, "/opt/trn_rl_repo")

import ml_dtypes
import numpy as np

import concourse.bass as bass
import concourse.mybir as mybir
import concourse.tile as tile
from concourse import bacc
from concourse.bass_utils import run_bass_kernel_spmd

# Problem shapes
B, S, DIM = 1, 2048, 2048
NH, NKV, HD = 32, 8, 64
NREP = NH // NKV
SCALE = 1.0 / float(np.sqrt(HD))
NCORES = 8
HPC = NH // NCORES            # query heads per core (4)
QKV = HPC * HD + 2 * HD       # fused qkv output dim per core (384)
QW = HPC * HD                 # query width per core (256)
SB = 512                      # seq block (attention q-block)
NSB = S // SB                 # 4
NT = S // 128                 # 16 seq tiles
ND = DIM // 128               # 16 contraction tiles
MYS = S // NCORES             # output rows per core (256)

F32 = mybir.dt.float32
BF16 = mybir.dt.bfloat16
I16 = mybir.dt.int16
F8 = mybir.dt.float8e4

# bf16 Schraudolph exp constants: exp(x) ~= bitcast_bf16(int16(A*x + B)).
# One DVE pass: the fp32->int16 output conversion rounds the affine result
# directly into bf16's bit layout (max rel err ~3.3%).
S16_A = float(128.0 / np.log(2.0))
S16_B = float(127 * 128 - 5.5)

_cache = {}

last_exec_time_ns = None


def _install_ntff_shim():
    """Register the NTFF profile hook so trace=True yields exec_time_ns."""
    import types
    if "antenv.axon_hooks" in sys.modules:
        return
    import antenv
    mod = types.ModuleType("antenv.axon_hooks")
    mod._hook = None
    mod.set_axon_ntff_profile_hook = lambda h: setattr(mod, "_hook", h)
    mod.get_axon_ntff_profile_hook = lambda: mod._hook
    sys.modules["antenv.axon_hooks"] = mod
    antenv.axon_hooks = mod
    from trn_agent_boot.trn_boot import _ntff_profile_via_ctypes
    hook = _ntff_profile_via_ctypes("/opt/axon/libaxon_pjrt.so")
    if hook is not None:
        mod._hook = hook


def _build():
    nc = bacc.Bacc("TRN2", target_bir_lowering=False, debug=False,
                   num_devices=NCORES)

    # Input staging happens in declaration order; order by first use:
    # qkv weights + x^T feed phase B immediately, rope consts next,
    # attention consts after, woT (phase D) last.
    qkvb_e = nc.declare_dram_parameter("qkvb", [1, QKV], BF16, isOutput=False)
    wqkvT_e = nc.declare_dram_parameter("wqkvT", [128, ND * QKV], BF16, isOutput=False)
    xT_e = nc.declare_dram_parameter("xT", [128, NSB * ND * SB], BF16, isOutput=False)
    cosd_e = nc.declare_dram_parameter("cosd", [128, NT * HD], F32, isOutput=False)
    nsin_e = nc.declare_dram_parameter("nsin", [128, NT * HD // 2], F32, isOutput=False)
    psin_e = nc.declare_dram_parameter("psin", [128, NT * HD // 2], F32, isOutput=False)
    ident_e = nc.declare_dram_parameter("ident", [128, 128], BF16, isOutput=False)
    mask_e = nc.declare_dram_parameter("masks", [128, 4 * SB], BF16, isOutput=False)
    esg_e = nc.declare_dram_parameter("esg", [8, 2], F32, isOutput=False)
    wob_e = nc.declare_dram_parameter("wob", [1, DIM], BF16, isOutput=False)
    woT_e = nc.declare_dram_parameter("woT", [128, ND * DIM], BF16, isOutput=False)
    out_e = nc.declare_dram_parameter("out", [MYS, DIM], F32, isOutput=True)

    with tile.TileContext(nc) as tc:
        with tc.tile_pool(name="const", bufs=1) as cp, \
             tc.tile_pool(name="rope", bufs=2) as rp, \
             tc.tile_pool(name="qkr", bufs=4) as qkrp, \
             tc.tile_pool(name="pt", bufs=5) as ptp, \
             tc.tile_pool(name="it", bufs=3) as itp, \
             tc.tile_pool(name="ep", bufs=3) as epp, \
             tc.tile_pool(name="fin", bufs=2) as fnp, \
             tc.tile_pool(name="dram", bufs=1, space="DRAM") as dp:

            # ---- staging (sync queue is FIFO: order = priority) ----
            qkvb_sb = cp.tile([1, QKV], BF16)
            nc.sync.dma_start(qkvb_sb[:], qkvb_e[:])
            wqkvT_sb = cp.tile([128, ND, QKV], BF16)
            wqv = wqkvT_e[:].rearrange("p (o f) -> p o f", o=ND)
            nc.sync.dma_start(wqkvT_sb[:, 0:ND // 2, :], wqv[:, 0:ND // 2, :])
            nc.sync.dma_start(wqkvT_sb[:, ND // 2:, :], wqv[:, ND // 2:, :])
            # x^T view, half-block-major so each DMA is one contiguous
            # 8KB-per-partition descriptor: [128, s, half, d, c]
            xTv = xT_e[:].rearrange("p (s h d c) -> p s h d c",
                                    s=NSB, h=2, d=ND)
            # two fixed x^T half-block buffers, alternated: reusing the SAME
            # tiles forces WAR pacing so later half-block DMAs cannot race
            # ahead and steal startup DMA bandwidth from the critical loads
            xt2 = [cp.tile([128, ND, SB // 2], BF16, name=f"xt2_{i}")
                   for i in range(2)]
            nc.sync.dma_start(xt2[0][:, 0:ND // 2, :],
                              xTv[:, 0, 0, 0:ND // 2, :])
            nc.sync.dma_start(xt2[0][:, ND // 2:, :],
                              xTv[:, 0, 0, ND // 2:, :])
            # DMA engines round-robin every in-flight transfer, so gate the
            # non-critical loads on xt2[0]'s data: the first QKV consumer
            # then gets the full DMA bandwidth
            cos_sb = cp.tile([128, NT, HD], F32)
            nsin_sb = cp.tile([128, NT, HD // 2], F32)
            psin_sb = cp.tile([128, NT, HD // 2], F32)
            ident_sb = cp.tile([128, 128], BF16)
            gate_src = xt2[0][0:1, 0, 0:1]
            nc.vector.tensor_copy(xt2[1][0:1, 0, 0:1], gate_src)
            nc.vector.tensor_copy(cos_sb[0:1, 0, 0:1], gate_src)
            nc.vector.tensor_copy(nsin_sb[0:1, 0, 0:1], gate_src)
            nc.vector.tensor_copy(psin_sb[0:1, 0, 0:1], gate_src)
            nc.vector.tensor_copy(ident_sb[0:1, 0:1], gate_src)
            nc.sync.dma_start(xt2[1][:], xTv[:, 0, 1])
            nc.sync.dma_start(cos_sb[:], cosd_e[:].rearrange(
                "p (o f) -> p o f", o=NT))
            nc.sync.dma_start(nsin_sb[:], nsin_e[:].rearrange(
                "p (o f) -> p o f", o=NT))
            nc.sync.dma_start(psin_sb[:], psin_e[:].rearrange(
                "p (o f) -> p o f", o=NT))
            nc.sync.dma_start(ident_sb[:], ident_e[:])

            woT_sb = cp.tile([128, ND, DIM], BF16)
            ones_sb = cp.tile([1, 128], BF16)
            nc.gpsimd.memset(ones_sb[:], 1.0)

            # PE warm-up: a gapless burst of dummy matmuls during the
            # initial input DMAs releases the HAM clock throttle before
            # real work begins.
            warm_sb = cp.tile([128, 512], BF16)
            nc.gpsimd.memset(warm_sb[:], 0.0)
            with tc.tile_pool(name="warm", bufs=1, space="PSUM") as wpp:
                warm_ps = wpp.tile([128, 512], F32, tag="warm")
                for _ in range(10):
                    nc.tensor.matmul(warm_ps[:], warm_sb[:, 0:128],
                                     warm_sb[:], start=True, stop=True)

            # persistent activations: q head-pairs packed in partition
            # halves, k duplicated into both halves (tile_position packing)
            qP = [cp.tile([128, S], BF16, name=f"qP{g}") for g in range(HPC // 2)]
            kTd = cp.tile([128, S], BF16)
            v_sb = cp.tile([128, NT, HD + 1], BF16)
            nc.gpsimd.memset(v_sb[:, :, HD:HD + 1], 1.0)
            # attention outputs per head: rows 0..HD-1 hold the (unnormalized
            # until the epilogue) PV output, row HD holds the softmax row sum
            oT5 = [cp.tile([HD + 1, S], BF16, name=f"oT5{h}")
                   for h in range(HPC)]
            # row-sum collectors (one per head pair): row r = 2*s + z
            srg8 = [cp.tile([8, SB], BF16, name=f"srg8_{g}")
                    for g in range(2)]
            rgb1 = cp.tile([1, 8 * SB], BF16)
            rbcz = [cp.tile([HD, S], BF16, name=f"rbcz{z}") for z in range(2)]
            # A2A gather tiles, 4 contraction strips each (phase D weights)
            agh = [[cp.tile([128, 4, MYS], BF16, name=f"ag{g}_{h}")
                    for h in range(2)] for g in range(2)]

            a2a_in = [dp.tile([S // 2, MYS], BF16, name=f"a2ai{g}")
                      for g in range(2)]
            a2a_out = [dp.tile([S // 2, MYS], BF16, name=f"a2ao{g}")
                       for g in range(2)]

            # ---- phase B: QKV projections + rope + transposes ----
            with tc.tile_pool(name="ppB", bufs=2, space="PSUM") as ppB:
                first_warm = [True]
                for s in range(NSB):
                    qkr_tiles = []
                    for tt in range(4):
                        t = 4 * s + tt
                        hb = 2 * s + tt // 2
                        if tt % 2 == 0 and hb >= 2:
                            # refill the alternating half-block buffer (the
                            # first two DMAs were issued at staging time)
                            nc.sync.dma_start(xt2[hb % 2][:],
                                              xTv[:, hb // 2, hb % 2])
                        xt = xt2[hb % 2]
                        xsl = (tt % 2) * 128

                        if first_warm[0]:
                            # dependency-gated warm-up: fires as soon as the
                            # first x^T tile lands, so QKV starts at the warm
                            # PE clock
                            first_warm[0] = False
                            warm2_ps = ppB.tile([128, 512], F32, tag="warmB",
                                                bufs=1)
                            for _ in range(12):
                                nc.tensor.matmul(warm2_ps[:],
                                                 xt[:, 0, 0:128],
                                                 warm_sb[:], start=True,
                                                 stop=True)

                        # fused qkv projection for this seq tile
                        acc = ppB.tile([128, QKV], F32, tag="acc", bufs=3)
                        for d in range(ND):
                            nc.tensor.matmul(acc[:],
                                             xt[:, d, xsl:xsl + 128],
                                             wqkvT_sb[:, d, :],
                                             start=(d == 0), stop=False)
                        nc.tensor.matmul(acc[:], ones_sb[0:1, :], qkvb_sb[:],
                                         start=False, stop=True)

                        # rope on q and k halves (free-dim ops, 5 = 4q+1k)
                        W = QW + HD  # 320
                        tmp = rp.tile([128, W], F32, tag="tmp")
                        qkr = qkrp.tile([128, W], BF16, tag="qkr")
                        acc5 = acc[:, 0:W].rearrange("p (h x) -> p h x", x=HD)
                        tmp5 = tmp[:].rearrange("p (h x) -> p h x", x=HD)
                        nc.vector.tensor_tensor(
                            tmp5[:, :, 0:HD // 2], acc5[:, :, HD // 2:HD],
                            nsin_sb[:, t:t + 1, :].to_broadcast(
                                [128, 5, HD // 2]),
                            mybir.AluOpType.mult)
                        nc.vector.tensor_tensor(
                            tmp5[:, :, HD // 2:HD], acc5[:, :, 0:HD // 2],
                            psin_sb[:, t:t + 1, :].to_broadcast(
                                [128, 5, HD // 2]),
                            mybir.AluOpType.mult)
                        nc.vector.tensor_tensor(
                            qkr[:], acc[:, 0:W],
                            cos_sb[:, t:t + 1, :].to_broadcast([128, 5, HD]),
                            mybir.AluOpType.mult)
                        nc.vector.tensor_tensor(qkr[:], qkr[:], tmp[:],
                                                mybir.AluOpType.add)
                        qkr_tiles.append(qkr)
                        # v: plain copy (cast to bf16)
                        nc.scalar.copy(v_sb[:, t, 0:HD], acc[:, QW + HD:QKV])

                    # transpose rope'd q/k for the block into [hd, seq]
                    for h in range(HPC + 1):
                        tq_ps = ppB.tile([HD, 512], BF16, tag="tq", bufs=2)
                        for tt in range(4):
                            nc.tensor.transpose(
                                tq_ps[:, tt * 128:(tt + 1) * 128],
                                qkr_tiles[tt][:, h * HD:(h + 1) * HD],
                                ident_sb[:])
                        sl = slice(s * SB, (s + 1) * SB)
                        if h < HPC:
                            dst = qP[h // 2][(h % 2) * HD:(h % 2) * HD + HD, sl]
                            if h % 2 == 0:
                                nc.scalar.copy(dst, tq_ps[:])
                            else:
                                nc.vector.tensor_copy(dst, tq_ps[:])
                        else:
                            nc.scalar.copy(kTd[0:HD, sl], tq_ps[:])
                            nc.vector.tensor_copy(kTd[HD:2 * HD, sl], tq_ps[:])

            # attention/output consts: needed only after phase B
            mask_sb = cp.tile([128, 4, SB], BF16)
            nc.sync.dma_start(mask_sb[:], mask_e[:].rearrange(
                "p (d f) -> p d f", d=4))
            esg_sb = cp.tile([8, 2], F32)
            nc.sync.dma_start(esg_sb[:], esg_e[:])
            wob_sb = cp.tile([1, DIM], BF16)
            nc.sync.dma_start(wob_sb[:], wob_e[:])

            # ---- phase C: attention (flash-style over causal k-tiles) ----
            # Scores for a head PAIR run concurrently on the two 64-row
            # halves of the PE array (tile_position row packing). Each k-tile
            # produces one [128, (z, 512)] fp32 score tile; exp/mask run as
            # single wide ops covering both heads; PV matmuls trail the
            # score stream by two k-tiles so the exp latency stays hidden.
            exp_counter = [0]
            mask_counter = [0]

            def emit_sums(g, s_lo, s_hi):
                # reciprocal softmax normalizers for seq blocks s_lo..s_hi-1
                # of head pair g, broadcast into rbcz via DRAM
                r0, r1 = 2 * s_lo, 2 * s_hi
                c0, c1 = s_lo * SB, s_hi * SB
                ns = s_hi - s_lo
                # engine APs must start 32-aligned, so compute on all 8 rows
                # (DVE cost depends on free size only) and slice the DMA
                srgf = epp.tile([8, SB], F32, tag="srgf", bufs=2)
                nc.vector.tensor_scalar(
                    srgf[:], srg8[g][:], esg_sb[:, g:g + 1],
                    None, mybir.AluOpType.add)
                rr = epp.tile([8, SB], F32, tag="rr", bufs=2)
                nc.vector.reciprocal_approx_fast(rr[:], srgf[:])
                rgb = epp.tile([8, SB], BF16, tag="rgb", bufs=2)
                nc.vector.tensor_copy(rgb[:], rr[:])
                # flatten rows r0:r1 into rgb1 cols (row-major: (s, z, c))
                nc.sync.dma_start(rgb1[0:1, r0 * SB:r1 * SB], rgb[r0:r1, :])
                rg1v = rgb1[0:1, r0 * SB:r1 * SB].rearrange(
                    "a (s z c) -> a z s c", z=2, c=SB)
                for z in range(2):
                    # GPS queue is empty at epilogue time, so the one-hop
                    # partition_broadcast beats a DRAM round trip
                    nc.gpsimd.partition_broadcast(
                        rbcz[z][:, c0:c1].rearrange("p (s c) -> p s c", c=SB),
                        rg1v[:, z])

            def emit_norm(g, s_lo, s_hi):
                # normalize + scatter seq blocks s_lo..s_hi-1 of head pair g
                c0, c1 = s_lo * SB, s_hi * SB
                ns = s_hi - s_lo
                for z in range(2):
                    h = 2 * g + z
                    nc.vector.tensor_tensor(oT5[h][0:HD, c0:c1],
                                            oT5[h][0:HD, c0:c1],
                                            rbcz[z][:, c0:c1],
                                            mybir.AluOpType.mult)
                    nc.sync.dma_start(
                        a2a_in[g][:].rearrange(
                            "(j two p) n -> two p j n",
                            j=NCORES, two=2)[z][:, 2 * s_lo:2 * s_hi, :],
                        oT5[h][0:HD, c0:c1].rearrange(
                            "p (j n) -> p j n", j=2 * ns))

            def emit_chunk(g):
                nc.gpsimd.collective_compute(
                    "AllToAll", mybir.AluOpType.bypass,
                    replica_groups=[list(range(NCORES))],
                    ins=[a2a_in[g].opt()], outs=[a2a_out[g].opt()])

            def emit_gathers(g):
                # phase D gather tiles ride the (otherwise idle during g1)
                # GpSimd queue: parking there blocks nothing
                a2o = a2a_out[g][:].rearrange("(c p) n -> c p n", p=128)
                for h in range(2):
                    nc.gpsimd.dma_start(
                        agh[g][h][:],
                        a2o[4 * h:4 * h + 4].rearrange("c p n -> p c n"))

            with tc.tile_pool(name="ppC", bufs=1, space="PSUM") as ppC:
                for g in range(HPC // 2):
                    # s descending: the tail block (last before the A2A
                    # chunk fires) is the smallest one
                    for s in reversed(range(NSB)):
                        n_kt = 4 * (s + 1)
                        sl = slice(s * SB, (s + 1) * SB)
                        pv = [ppC.tile([HD + 1, 512], F32, tag=f"pv{z}",
                                       bufs=1, name=f"pv{g}_{s}_{z}")
                              for z in range(2)]
                        pts = []

                        def emit_pv(i0, i1):
                            # PV for k-tiles i0..i1 (inclusive), grouped by
                            # PSUM bank (z)
                            for z in range(2):
                                for ii in range(i0, i1 + 1):
                                    nc.tensor.matmul(
                                        pv[z][:], v_sb[:, ii, :],
                                        pts[ii][:, z, :],
                                        start=(ii == 0),
                                        stop=(ii == n_kt - 1))

                        for i in range(n_kt):
                            sc = ppC.tile([128, 2, 512], F32, tag="sc",
                                          bufs=3)
                            for z in range(2):
                                nc.tensor.matmul(
                                    sc[:, z, :],
                                    kTd[z * HD:(z + 1) * HD,
                                        i * 128:(i + 1) * 128],
                                    qP[g][z * HD:(z + 1) * HD, sl],
                                    start=True, stop=True,
                                    tile_position=(z * HD, 0))
                            cnt = exp_counter[0]
                            exp_counter[0] += 1
                            if (cnt % 8) in (0, 1, 3, 4, 6):
                                pt = ptp.tile([128, 2, 512], BF16, tag="pt")
                                nc.scalar.activation(
                                    pt[:], sc[:],
                                    mybir.ActivationFunctionType.Exp,
                                    scale=SCALE)
                                ptv = pt[:]
                            else:
                                # single-pass bf16 Schraudolph exp: the
                                # int16 affine result IS the bf16 bits
                                it_ = itp.tile([128, 2, 512], I16, tag="it")
                                nc.vector.tensor_scalar(
                                    it_[:], sc[:], S16_A * SCALE, S16_B,
                                    mybir.AluOpType.mult,
                                    mybir.AluOpType.add)
                                ptv = it_[:].bitcast(BF16)
                            if i >= 4 * s:
                                # causal mask for a diagonal k-tile, one op
                                # covering both heads; mostly DVE, some GPS
                                d = i - 4 * s
                                mc = mask_counter[0]
                                mask_counter[0] += 1
                                # GPS masks only during g0: in g1 the GPS
                                # FIFO must stay clear for the A2A triggers
                                eng = (nc.gpsimd
                                       if (g == 0 and s != 3
                                           and (mc % 4) == 0)
                                       else nc.vector)
                                eng.tensor_tensor(
                                    ptv, ptv,
                                    mask_sb[:, d:d + 1, :].to_broadcast(
                                        [128, 2, 512]),
                                    mybir.AluOpType.mult)
                            pts.append(ptv)
                            # PV trails by 2 k-tiles so exp latency is
                            # covered by the next scores
                            if i >= 3 and i % 2 == 1:
                                emit_pv(i - 3, i - 2)
                        emit_pv(n_kt - 2, n_kt - 1)

                        for z in range(2):
                            # pv -> persistent bf16 output rows (incl. the
                            # sum row at partition HD); sum row then hops to
                            # the srg8 collector via DMA
                            h = 2 * g + z
                            if z == 0:
                                nc.scalar.copy(oT5[h][:, sl], pv[z][:])
                            else:
                                nc.vector.tensor_copy(oT5[h][:, sl], pv[z][:])
                            nc.sync.dma_start(
                                srg8[g][2 * s + z:2 * s + z + 1, :],
                                oT5[h][HD:HD + 1, sl])

                        if g == 0 and s == 3:
                            # stream woT in now: HBM is otherwise idle during
                            # attention and phase D needs it much later. The
                            # gate copy adds a dependency on attention output
                            # so the scheduler cannot hoist the load into the
                            # startup HBM rush.
                            nc.gpsimd.tensor_copy(
                                woT_sb[0:1, 0, 0:1],
                                oT5[2 * g][0:1, 3 * SB:3 * SB + 1])
                            nc.gpsimd.dma_start(
                                woT_sb[:, 0:ND // 2, :],
                                woT_e[:, 0:ND // 2 * DIM].rearrange(
                                    "p (o f) -> p o f", o=ND // 2))
                        if g == 0 and s == 2:
                            nc.gpsimd.tensor_copy(
                                woT_sb[0:1, ND // 2, 0:1],
                                oT5[2 * g][0:1, 2 * SB:2 * SB + 1])
                            nc.gpsimd.dma_start(
                                woT_sb[:, ND // 2:, :],
                                woT_e[:, ND // 2 * DIM:].rearrange(
                                    "p (o f) -> p o f", o=ND // 2))
                        # pair-0 epilogues are deferred into g1's blocks so
                        # their semaphore waits never park an engine FIFO
                        # ahead of available attention work; pair-1's batchA
                        # chain starts one block early for the same reason
                        if g == 1 and s == 3:
                            emit_sums(0, 1, NSB)
                            emit_norm(0, 1, NSB)
                        if g == 1 and s == 2:
                            emit_sums(0, 0, 1)
                            emit_norm(0, 0, 1)
                            emit_chunk(0)
                        if g == 1 and s == 1:
                            emit_sums(1, 1, NSB)
                            emit_gathers(0)
                # pair-1 epilogue tail: the last chain gates the final A2A
                emit_norm(1, 1, NSB)
                emit_sums(1, 0, 1)
                emit_norm(1, 0, 1)
                emit_chunk(1)
                emit_gathers(1)

            # ---- output projection for my sequence slice ----
            # even contraction tiles (chunk 0) first so they can run while
            # A2A chunk 1 is still in flight
            with tc.tile_pool(name="ppD", bufs=1, space="PSUM") as ppD:
                fps = [ppD.tile([128, 512], F32, name=f"fp{m}_{n}", bufs=1)
                       for m in range(MYS // 128) for n in range(DIM // 512)]
                for parity in range(2):
                    for m in range(MYS // 128):
                        for n in range(DIM // 512):
                            fp = fps[m * (DIM // 512) + n]
                            for c in range(ND // 2):
                                kt = 2 * c + parity
                                nc.tensor.matmul(
                                    fp[:],
                                    agh[parity][c // 4][:, c % 4,
                                                        m * 128:(m + 1) * 128],
                                    woT_sb[:, kt, n * 512:(n + 1) * 512],
                                    start=(c == 0 and parity == 0),
                                    stop=(parity == 1 and c == ND // 2 - 1))
                            if parity == 0:
                                # bias needs no A2A data: accumulate it in
                                # the first wave, off the post-collective
                                # critical path
                                nc.tensor.matmul(
                                    fp[:], ones_sb[0:1, :],
                                    wob_sb[0:1, n * 512:(n + 1) * 512],
                                    start=False, stop=False)
                            if parity == 1:
                                fo = fnp.tile([128, 512], F32, tag="fo")
                                if (m * 4 + n) % 2 == 0:
                                    nc.scalar.copy(fo[:], fp[:])
                                else:
                                    nc.vector.tensor_copy(fo[:], fp[:])
                                nc.sync.dma_start(
                                    out_e[m * 128:(m + 1) * 128,
                                          n * 512:(n + 1) * 512], fo[:])

    nc.compile()
    return nc


def _host_prep(x, rope_cache, wq_w, wq_b, wk_w, wk_b, wv_w, wv_b,
               wo_w, wo_b, sinks):
    """Build the per-core input maps (sharding + layout prep)."""
    # x^T, half-block-major: [128, (s half d c)] with s=block, half=half-
    # block, d=dim-strip, c=col — each half-block contiguous per partition
    xT = np.asarray(x, np.float32).reshape(S, DIM).T.astype(
        ml_dtypes.bfloat16)
    xTb = np.ascontiguousarray(
        xT.reshape(ND, 128, NSB, 2, SB // 2).transpose(1, 2, 3, 0, 4).reshape(
            128, NSB * ND * SB))

    def _pm(a):
        # [S, F] -> [128, (S//128) * F] partition-major packing
        f = a.shape[1]
        return np.ascontiguousarray(
            a.reshape(S // 128, 128, f).transpose(1, 0, 2).reshape(
                128, (S // 128) * f))

    cos = np.asarray(rope_cache[:, :HD // 2], np.float32)
    sin = np.asarray(rope_cache[:, HD // 2:], np.float32)
    cosd = _pm(np.concatenate([cos, cos], axis=1))
    nsin = _pm(-sin)
    psin = _pm(sin)
    # causal masks for the 4 diagonal 128-row k-tiles of a 512-col q block
    masks = np.zeros((4, 128, SB), np.float32)
    for d in range(4):
        for p in range(128):
            masks[d, p, d * 128 + p:] = 1.0
    masks = np.ascontiguousarray(
        masks.transpose(1, 0, 2).reshape(128, 4 * SB)).astype(
            ml_dtypes.bfloat16)
    ident = np.eye(128, dtype=ml_dtypes.bfloat16)
    woT = np.asarray(wo_w, np.float32).T.astype(ml_dtypes.bfloat16)
    woT = np.ascontiguousarray(
        woT.reshape(ND, 128, DIM).transpose(1, 0, 2).reshape(128, ND * DIM))
    wob = np.asarray(wo_b, np.float32).astype(
        ml_dtypes.bfloat16).reshape(1, DIM)
    es_all = np.exp(np.asarray(sinks, np.float64)).astype(np.float32)

    in_maps = []
    for c in range(NCORES):
        qsl = slice(c * QW, (c + 1) * QW)
        ksl = slice(c * HD, (c + 1) * HD)
        wqkvT = np.concatenate([
            np.asarray(wq_w, np.float32)[qsl].T,
            np.asarray(wk_w, np.float32)[ksl].T,
            np.asarray(wv_w, np.float32)[ksl].T],
            axis=1).astype(ml_dtypes.bfloat16)
        wqkvT = np.ascontiguousarray(
            wqkvT.reshape(ND, 128, QKV).transpose(1, 0, 2).reshape(
                128, ND * QKV))
        qkvb = np.ascontiguousarray(np.concatenate([
            np.asarray(wq_b, np.float32)[qsl],
            np.asarray(wk_b, np.float32)[ksl],
            np.asarray(wv_b, np.float32)[ksl]]).astype(
                ml_dtypes.bfloat16)).reshape(1, QKV)
        # esg[2s+z, g] = exp(sink[4c + 2g + z]) (replicated over s)
        esg = np.zeros((8, 2), np.float32)
        for gg in range(2):
            for z in range(2):
                esg[z::2, gg] = es_all[4 * c + 2 * gg + z]
        in_maps.append({
            "xT": xTb, "wqkvT": wqkvT, "qkvb": qkvb, "cosd": cosd,
            "nsin": nsin, "psin": psin, "masks": masks, "ident": ident,
            "woT": woT, "wob": wob, "esg": esg,
        })
    return in_maps


def kernel(**inputs):
    global last_exec_time_ns
    if "nc" not in _cache:
        _cache["nc"] = _build()
    nc = _cache["nc"]
    in_maps = _host_prep(**inputs)
    trace = bool(int(os.environ.get("BASS_KERNEL_TRACE", "0")))
    if trace:
        try:
            _install_ntff_shim()
        except Exception:
            trace = False
    tc_env = os.environ.get("BASS_KERNEL_TRACE_CORES")
    kw = {}
    if trace and tc_env:
        kw["trace_cores"] = [int(c) for c in tc_env.split(",")]
    res = run_bass_kernel_spmd(nc, in_maps, core_ids=list(range(NCORES)),
                               trace=trace, **kw)
    last_exec_time_ns = res.exec_time_ns
    out = np.concatenate([res.results[c]["out"] for c in range(NCORES)],
                         axis=0)
    return out.reshape(B, S, NH * HD)

